# revision 1
# baseline (speedup 1.0000x reference)
"""COIL sparse-attention scoring kernel for 8 Trainium2 NeuronCores.

Strategy
--------
Shard the doc axis (Bd=128) across the 8 cores (16 docs each); qry tensors are
replicated. The exact-token-match mask is folded INTO the matmul: each token id
(vocab 1000) is encoded as three base-10 digit one-hots scaled by ALPHA=32 and
appended to the reps. Then

    v[qs, ct] = <qry_ext[qs], doc_ext[ct]> = S[qs, ct] + 1024 * match_digits

where match_digits == 3 iff the ids are equal, so

    tok[qs, c] = relu(max_t v[qs, c, t] - 3072)

reproduces the reference masked-max exactly (non-match scores sit below 2100,
matches above 3000). The qry reps are split hi/lo in bf16 (3 cross terms) so
the matmul runs at full bf16 rate with ~fp32 accuracy: K = 3*32 + 30 = 126.

Per core: 32 q-tiles of 128 q-positions; each q-tile is one [126,128]x[126,2048]
matmul into PSUM [128, 2048]. The per-doc max over the 128 doc tokens is split
between the DVE (direct tensor_reduce from PSUM) and a ScalarE relu-convert to
fp16 followed by a DVE tensor_tensor max tree at 2x rate. The sum over query
positions is a ones-vector matmul on the PE (partition-dim sum). CLS scores,
the tiny 4-way q-tile fold, and the final max over the 8 query chunks are done
on host (a few thousand elements).
"""

import os
import numpy as np
import ml_dtypes

Bq, Sq, Bd, Sd, D, Dc = 8, 512, 128, 128, 32, 768
NCORES = 8
BD_PER = Bd // NCORES          # 16 docs per core
K_EXT = 126                    # 32*3 rep dims + 30 one-hot dims
SQF = Bq * Sq                  # 4096 query positions
DCOL = BD_PER * Sd             # 2048 doc tokens per core
NQT = SQF // 128               # 32 q-tiles
ALPHA = 32.0
OFF = 3.0 * ALPHA * ALPHA      # 3072: offset of a full 3-digit match
# q-tile qt goes to the DVE-direct path iff qt % DIRECT_PERIOD == DIRECT_PERIOD-1;
# the rest go ScalarE-relu-fp16 -> DVE max tree. Whole-tile assignment keeps each
# PSUM tile single-reader (fewer semaphore waits).
DIRECT_PERIOD = int(os.environ.get("KERNEL_DIRECT_PERIOD", "4"))
TREE_LEVELS = int(os.environ.get("KERNEL_TREE_LEVELS", "3"))
# 6 warm-up MMs (~3.8us busy) sits right at the HAM 3.4us flip threshold and
# is bimodal run-to-run (71.6 vs 75.5 measured); 12 is ~0.5us slower at best
# but stable across runs.
WARMUP_MMS = int(os.environ.get("KERNEL_WARMUP_MMS", "12"))
BRIDGE_MMS = int(os.environ.get("KERNEL_BRIDGE_MMS", "0"))
# prune query positions whose token id does not appear in this core's doc
# slab (they contribute exactly 0): compact to NQT_PRUNED q-tiles per core
PRUNE = os.environ.get("KERNEL_PRUNE", "1") == "1"
NQT_PRUNED = int(os.environ.get("KERNEL_NQT_PRUNED", "29"))
# fraction knob: tree q-tiles where GPSIMD runs the first max-tree level
# instead of the DVE. Disabled: this walrus build rejects TensorTensor on
# the Pool engine ("Instruction engine check failed").
GPSIMD_TT1_MOD = int(os.environ.get("KERNEL_GPSIMD_TT1_MOD", "0"))

_CACHE = {}


def _bf16(x):
    return x.astype(ml_dtypes.bfloat16)


def _onehot_digits(ids):
    """ids [N] int in [0,1000) -> [N,30] base-10 digit one-hot (float32)."""
    n = ids.shape[0]
    H = np.zeros((n, 30), dtype=np.float32)
    r = np.arange(n)
    H[r, ids % 10] = 1.0
    H[r, 10 + (ids // 10) % 10] = 1.0
    H[r, 20 + ids // 100] = 1.0
    return H


def _build_qry_ext(qry_reps, qry_input_ids, qry_attention_mask):
    q = np.asarray(qry_reps, np.float32).reshape(SQF, D)
    ids = np.asarray(qry_input_ids, np.int64).reshape(SQF)
    q_hi = _bf16(q).astype(np.float32)
    q_lo = _bf16(q - q_hi).astype(np.float32)
    H = ALPHA * _onehot_digits(ids)
    ext = np.concatenate([q_hi, q_lo, q_hi, H], axis=1)  # [SQF, 126]
    # rows that must contribute 0: CLS (s=0), SEP (last attended pos), mask==0
    mask = np.asarray(qry_attention_mask, np.int64).copy()
    sep = mask.sum(axis=1) - 1
    mask[np.arange(Bq), sep] = 0
    mask[:, 0] = 0
    ext *= mask.reshape(SQF, 1).astype(np.float32)
    return np.ascontiguousarray(_bf16(ext).T)  # [126, SQF]


def _build_doc_ext(doc_reps, doc_input_ids):
    d = np.asarray(doc_reps, np.float32).reshape(-1, D)
    ids = np.asarray(doc_input_ids, np.int64).reshape(-1)
    d_hi = _bf16(d).astype(np.float32)
    d_lo = _bf16(d - d_hi).astype(np.float32)
    H = ALPHA * _onehot_digits(ids)
    ext = np.concatenate([d_hi, d_hi, d_lo, H], axis=1)  # [N, 126]
    return np.ascontiguousarray(_bf16(ext).T)  # [126, N]


_LDW_PATCHED = False


def _patch_ldw_opt():
    """bir_verify_and_optimise hardcodes --enable-ldw-opt=false, which makes
    walrus emit one LDWEIGHTS per matmul even when the stationary operand is
    unchanged (4x redundant here). Append =true (last flag wins)."""
    global _LDW_PATCHED
    # Tile emits standalone InstLdweights, which walrus's ldw-opt rejects;
    # keep this opt-in for experiments only.
    if _LDW_PATCHED or not os.environ.get("KERNEL_LDW_OPT"):
        return
    import concourse.bass_utils as bu

    orig = bu.get_walrus_args

    def patched(*a, **k):
        return orig(*a, **k) + ["--enable-ldw-opt=true"]

    bu.get_walrus_args = patched
    _LDW_PATCHED = True


def _split_multi_waits(nc, mybir):
    """This container's walrus accepts only ONE sync-wait per instruction
    ("Too many sync wait commands"). Hoist extra waits into standalone
    EventSemaphore instructions on the same engine right before the offender
    (the sequencer blocks on each in order — semantically identical)."""
    n = 0
    for func in nc.m.functions:
        for bb in func.blocks:
            out = []
            for inst in bb.instructions:
                si = inst.sync_info
                if si is not None and len(si.on_wait) > 1:
                    waits = list(si.on_wait)
                    for w in waits[:-1]:
                        n += 1
                        out.append(
                            mybir.InstEventSemaphore(
                                name=f"W-{inst.name}-{n}",
                                engine=inst.engine,
                                ins=[],
                                outs=[],
                                debug=inst.debug,
                                sync_info=mybir.SyncInfo(
                                    on_wait=[w], on_update=[]
                                ),
                            )
                        )
                    inst.sync_info = mybir.SyncInfo(
                        on_wait=[waits[-1]], on_update=list(si.on_update)
                    )
                out.append(inst)
            bb.instructions = out
    return n


def _groups(nqt):
    """Final-sum groups: up to 8 q-tiles share one selector matmul (the
    off-diagonal blocks of the [8G, 16G] product are computed but unused)."""
    return [range(g, min(g + 8, nqt)) for g in range(0, nqt, 8)]


def _build_nc(direct_period, tree_levels, nqt):
    import concourse.bass as bass
    import concourse.mybir as mybir
    import concourse.tile as tile
    from concourse.bass import ts

    bf16, f16, f32 = mybir.dt.bfloat16, mybir.dt.float16, mybir.dt.float32
    nc = bass.Bass("TRN2", target_bir_lowering=False, debug=False)
    sqf = nqt * 128
    qryT = nc.dram_tensor("qryT", [K_EXT, sqf], bf16, kind="ExternalInput").ap()
    docT = nc.dram_tensor("docT", [K_EXT, DCOL], bf16, kind="ExternalInput").ap()
    selT = nc.dram_tensor("selT", [128, 8 * nqt], f32, kind="ExternalInput").ap()
    out = nc.dram_tensor("out", [64, 16 * nqt], f32, kind="ExternalOutput").ap()

    phase = int(os.environ.get("KERNEL_DIRECT_PHASE", "0"))
    is_direct = [
        direct_period > 0 and qt % direct_period == phase % direct_period
        for qt in range(nqt)
    ]
    n_direct = sum(is_direct)
    with tile.TileContext(nc) as tc:
        with (
            tc.tile_pool(name="inp", bufs=1) as inp,
            tc.tile_pool(name="psum", bufs=2, space="PSUM") as psum,
            tc.tile_pool(name="stage", bufs=3) as stp,
            tc.tile_pool(name="tree", bufs=2) as trp,
            tc.tile_pool(name="accp", bufs=1) as accp,
        ):
            # PE warm-up: ~3.5us of junk matmuls during the DMA head so the
            # HAM clock-gate reaches 8/8 before the real work starts
            scratch = inp.tile([K_EXT, 512], bf16)
            nc.vector.memset(scratch[:], 0.0)
            wps = psum.tile([128, 512], f32, tag="score")
            for _ in range(WARMUP_MMS):
                nc.tensor.matmul(
                    wps[:], scratch[:, 0:128], scratch[:], start=True, stop=True
                )

            # doc chunk 0 + qry chunk 0 first so q-tile 0 can start early
            qry_sb = inp.tile([K_EXT, sqf], bf16)
            doc_sb = inp.tile([K_EXT, DCOL], bf16)
            sel_sb = inp.tile([128, 8 * nqt], f32)
            # first chunks split across the HWDGE (sync) and SWDGE (gpsimd)
            # queues so they land in parallel instead of serializing
            nc.sync.dma_start(doc_sb[:, ts(0, 512)], docT[:, ts(0, 512)])
            nc.gpsimd.dma_start(qry_sb[:, ts(0, 512)], qryT[:, ts(0, 512)])
            nc.sync.dma_start(doc_sb[:, ts(1, 512)], docT[:, ts(1, 512)])
            nc.gpsimd.dma_start(doc_sb[:, ts(2, 512)], docT[:, ts(2, 512)])
            nc.sync.dma_start(doc_sb[:, ts(3, 512)], docT[:, ts(3, 512)])
            for off in range(512, sqf, 512):
                w = min(512, sqf - off)
                nc.sync.dma_start(qry_sb[:, off : off + w], qryT[:, off : off + w])
            nc.sync.dma_start(sel_sb[:], selT[:])

            accum = accp.tile([128, 16 * nqt], f32)
            draw = accp.tile([128, 16 * max(n_direct, 1)], f32)
            negoff = accp.tile([128, 1], f32)
            nc.vector.memset(negoff[:], -OFF)

            di = 0
            for qt in range(nqt):
                ps = psum.tile([128, DCOL], f32, tag="score")
                for j in range(DCOL // 512):
                    nc.tensor.matmul(
                        ps[:, ts(j, 512)],
                        qry_sb[:, ts(qt, 128)],
                        doc_sb[:, ts(j, 512)],
                        start=True,
                        stop=True,
                    )
                if is_direct[qt]:
                    # whole tile on DVE straight from PSUM (raw v scale),
                    # then tok = max(raw, OFF) - OFF into the accum cols
                    nc.vector.reduce_max(
                        draw[:, di * 16 : (di + 1) * 16],
                        ps[:].rearrange("p (c t) -> p c t", t=Sd),
                        axis=mybir.AxisListType.X,
                    )
                    nc.vector.tensor_scalar(
                        accum[:, qt * 16 : (qt + 1) * 16],
                        draw[:, di * 16 : (di + 1) * 16],
                        OFF,
                        -OFF,
                        mybir.AluOpType.max,
                        mybir.AluOpType.add,
                    )
                    di += 1
                else:
                    # fp16 relu(v - OFF) on ScalarE; tree then maxes toks
                    st = stp.tile([128, BD_PER * Sd], f16, tag="stage")
                    nc.scalar.activation(
                        st[:],
                        ps[:],
                        mybir.ActivationFunctionType.Relu,
                        bias=negoff[:],
                    )
                    cur, width = st, Sd
                    for lev in range(tree_levels):
                        nxt = trp.tile([128, BD_PER * width // 2], f16, tag=f"t{lev}")
                        cv = cur[:].rearrange("p (c t) -> p c t", t=width)
                        eng = (
                            nc.gpsimd
                            if (
                                lev == 0
                                and GPSIMD_TT1_MOD > 0
                                and qt % GPSIMD_TT1_MOD == 0
                            )
                            else nc.vector
                        )
                        eng.tensor_max(
                            nxt[:].rearrange("p (c t) -> p c t", t=width // 2),
                            cv[:, :, 0 : width // 2],
                            cv[:, :, width // 2 : width],
                        )
                        cur, width = nxt, width // 2
                    nc.vector.reduce_max(
                        accum[:, qt * 16 : (qt + 1) * 16],
                        cur[:].rearrange("p (c t) -> p c t", t=width),
                        axis=mybir.AxisListType.X,
                    )
            # a few junk matmuls with late priority: the scheduler runs them
            # when the PE idles after the last q-tile, keeping the HAM clock
            # warm for the final partition-sum matmuls
            for _ in range(BRIDGE_MMS):
                bp = psum.tile([128, 512], f32, tag="score")
                nc.tensor.matmul(
                    bp[:], scratch[:, 0:128], scratch[:], start=True, stop=True
                )
            # per-q partition sums: for each group of up to 8 q-tiles, one
            # matmul with the q-membership selector as the stationary operand;
            # only the [8,16] diagonal blocks are used (host slices them out)
            osb = accp.tile([64, 16 * nqt], f32)
            nc.vector.memset(osb[:], 0.0)
            for g, grp in enumerate(_groups(nqt)):
                qts = list(grp)
                gn = len(qts)
                c0 = qts[0] * 16
                fin = psum.tile([8 * gn, 16 * gn], f32, tag="score")
                nc.tensor.matmul(
                    fin[:],
                    sel_sb[:, qts[0] * 8 : (qts[-1] + 1) * 8],
                    accum[:, c0 : c0 + 16 * gn],
                    start=True,
                    stop=True,
                )
                if g % 2 == 0:
                    nc.vector.tensor_copy(osb[0 : 8 * gn, c0 : c0 + 16 * gn], fin[:])
                else:
                    nc.scalar.copy(osb[0 : 8 * gn, c0 : c0 + 16 * gn], fin[:])
            nc.sync.dma_start(out[:], osb[:])
    _split_multi_waits(nc, mybir)
    return nc


def _get_nc(nqt):
    _patch_ldw_opt()
    key = (
        DIRECT_PERIOD,
        TREE_LEVELS,
        nqt,
        os.environ.get("KERNEL_DIRECT_PHASE", "0"),
    )
    if key not in _CACHE:
        _CACHE[key] = _build_nc(key[0], key[1], nqt)
    return _CACHE[key]


def _qry_row_mask(inputs):
    """[Bq, Sq] bool: rows that can contribute (attended, not CLS/SEP)."""
    mask = np.asarray(inputs["qry_attention_mask"], np.int64).copy()
    sep = mask.sum(axis=1) - 1
    mask[np.arange(Bq), sep] = 0
    mask[:, 0] = 0
    return mask.astype(bool)


def _prepare_in_maps(inputs):
    qT_full = _build_qry_ext(
        inputs["qry_reps"], inputs["qry_input_ids"], inputs["qry_attention_mask"]
    )  # [K_EXT, SQF] bf16
    doc_reps = np.asarray(inputs["doc_reps"], np.float32)
    doc_ids = np.asarray(inputs["doc_input_ids"], np.int64)
    qry_ids = np.asarray(inputs["qry_input_ids"], np.int64).reshape(SQF)
    row_ok = _qry_row_mask(inputs).reshape(SQF)
    qpos_q = np.repeat(np.arange(Bq), Sq)  # q index of each row

    nqt = NQT
    sels = None
    if PRUNE:
        sels = []
        for core in range(NCORES):
            sl = slice(core * BD_PER, (core + 1) * BD_PER)
            vocab = np.zeros(1000, dtype=bool)
            vocab[doc_ids[sl].reshape(-1)] = True
            keep = row_ok & vocab[qry_ids]
            sels.append(np.nonzero(keep)[0])
        if max(len(s) for s in sels) <= NQT_PRUNED * 128:
            nqt = NQT_PRUNED
        else:  # fallback: shapes must be compile-time fixed
            sels = None

    in_maps = []
    sqf = nqt * 128
    for core in range(NCORES):
        sl = slice(core * BD_PER, (core + 1) * BD_PER)
        dT = _build_doc_ext(doc_reps[sl], doc_ids[sl])
        if sels is not None:
            rows = sels[core]
            qT = np.zeros((K_EXT, sqf), dtype=qT_full.dtype)
            qT[:, : len(rows)] = qT_full[:, rows]
            qrow = qpos_q[rows]
        else:
            qT = qT_full
            qrow = qpos_q
        # selector: sel[p, qt*8+m] = 1 iff row qt*128+p belongs to query m
        sel = np.zeros((128, 8 * nqt), dtype=np.float32)
        for qt in range(nqt):
            seg = qrow[qt * 128 : (qt + 1) * 128]
            sel[np.arange(len(seg)), qt * 8 + seg] = 1.0
        in_maps.append({"qryT": qT, "docT": dT, "selT": sel})
    return in_maps, nqt


def _assemble(inputs, results, nqt):
    toks = np.zeros((Bq, Bd), dtype=np.float32)
    for core in range(NCORES):
        osb = np.asarray(results[core]["out"], np.float32)  # [64, 16*nqt]
        part = np.zeros((Bq, BD_PER), dtype=np.float32)
        for g, grp in enumerate(_groups(nqt)):
            for tl, qt in enumerate(grp):
                part += osb[8 * tl : 8 * tl + 8, qt * 16 : (qt + 1) * 16]
        toks[:, core * BD_PER : (core + 1) * BD_PER] = part
    cls = np.asarray(inputs["qry_cls"], np.float32) @ np.asarray(
        inputs["doc_cls"], np.float32
    ).T
    scores = toks + cls
    return scores.max(axis=0).reshape(-1).astype(np.float32)


def _ensure_ntff_hook():
    """This container's antenv lacks axon_hooks; synthesize the module and
    register the ctypes-based NTFF profile hook so trace=True works."""
    import sys
    import types

    if "antenv.axon_hooks" in sys.modules:
        return
    mod = types.ModuleType("antenv.axon_hooks")
    state = {"hook": None}
    mod.set_axon_ntff_profile_hook = lambda h: state.__setitem__("hook", h)
    mod.get_axon_ntff_profile_hook = lambda: state["hook"]
    sys.modules["antenv.axon_hooks"] = mod
    try:
        import antenv

        antenv.axon_hooks = mod
    except ImportError:
        pass
    try:
        from trn_agent_boot.trn_boot import _ntff_profile_via_ctypes

        mod.set_axon_ntff_profile_hook(
            _ntff_profile_via_ctypes("/opt/axon/libaxon_pjrt.so")
        )
    except Exception:
        pass


def run(inputs, trace=False, **kwargs):
    """Run on the 8 NeuronCores; returns (output, BassKernelResults)."""
    from concourse.bass_utils import run_bass_kernel_spmd

    if trace:
        _ensure_ntff_hook()
    in_maps, nqt = _prepare_in_maps(inputs)
    nc = _get_nc(nqt)
    res = run_bass_kernel_spmd(
        nc, in_maps, core_ids=list(range(NCORES)), trace=trace, **kwargs
    )
    return _assemble(inputs, res.results, nqt), res


def kernel(**inputs) -> np.ndarray:
    out, _ = run(inputs)
    return out



# revision 6
# speedup vs baseline: 2.3701x; 2.3701x over previous
"""COIL sparse-attention scoring kernel for 8 Trainium2 NeuronCores.

Strategy: vocab-range-blocked sparse scoring
--------------------------------------------
Shard the doc axis (Bd=128) across the 8 cores (16 docs each); qry tensors are
replicated. Only (q-token, doc-token) pairs with EQUAL ids contribute, so the
full [128 q, 2048 doc-token] cartesian per q-tile is 8x wasteful. Instead the
host sorts q-rows by token id and greedily packs contiguous vocab ranges into
tiles: each tile holds <=128 q-rows whose ids fall in a range of <=36 distinct
ids, and only the doc tokens with ids in that range, grouped per doc into
T=16 slots -> 16 docs x 16 slots = 256 columns per tile (vs 2048).

Exact-match detection stays folded into the matmul: each id is encoded by its
LOCAL index within the range as a 2-digit base-6 one-hot scaled by ALPHA=32 and
appended to the bf16 reps (K = 32 + 12 = 44). Then

    v[q, col] = S[q, col] + 1024 * match_digits,  match_digits == 2 iff equal,

so tok[q, doc] = relu(max_slots v - 2048) reproduces the reference masked max
(non-match v <= 1024 + |S|max ~ 1064, match v >= 2048 - |S|max ~ 2008).

Per core: ~29 tiles; one [44,128]x[44,256] matmul per tile into a shared PSUM
region of 8 tiles (4 banks); ONE DVE reduce_max (t=16) per 8-tile group
produces the per-doc maxes; a single ScalarE relu(x-OFF) pass converts them to
tok scores; per-q sums are ones/selector matmuls on the PE (partition-dim sum).
CLS scores and the final max over the 8 query chunks are done on host (tiny).
"""

import os
import numpy as np
import ml_dtypes

Bq, Sq, Bd, Sd, D, Dc = 8, 512, 128, 128, 32, 768
NCORES = 8
BD_PER = Bd // NCORES          # 16 docs per core
K_EXT = 44                     # 32 rep dims + 12 one-hot dims (2 digits base 6)
T_SLOTS = 16                   # doc-token slots per (doc, range)
C_TILE = BD_PER * T_SLOTS      # 256 doc columns per tile
MAXIDS = 36                    # max distinct ids per range (6*6 one-hot codes)
GSZ = 8                        # tiles per PSUM region (8 * 256 * 4B = 4 banks)
ALPHA = 32.0
OFF = 2.0 * ALPHA * ALPHA      # 2048: offset of a full 2-digit match
WARMUP_MMS = int(os.environ.get("KERNEL_WARMUP_MMS", "12"))

_CACHE = {}


def _bf16(x):
    return x.astype(ml_dtypes.bfloat16)


def _onehot6(idx):
    """local indices [N] in [0,36) -> [N,12] 2-digit base-6 one-hot (f32)."""
    idx = np.asarray(idx, np.int64)
    H = np.zeros((len(idx), 12), dtype=np.float32)
    r = np.arange(len(idx))
    H[r, idx % 6] = 1.0
    H[r, 6 + idx // 6] = 1.0
    return H


_LDW_PATCHED = False


def _patch_ldw_opt():
    """bir_verify_and_optimise hardcodes --enable-ldw-opt=false; opt-in knob."""
    global _LDW_PATCHED
    if _LDW_PATCHED or not os.environ.get("KERNEL_LDW_OPT"):
        return
    import concourse.bass_utils as bu

    orig = bu.get_walrus_args

    def patched(*a, **k):
        return orig(*a, **k) + ["--enable-ldw-opt=true"]

    bu.get_walrus_args = patched
    _LDW_PATCHED = True


def _split_multi_waits(nc, mybir):
    """This container's walrus accepts only ONE sync-wait per instruction
    ("Too many sync wait commands"). Hoist extra waits into standalone
    EventSemaphore instructions on the same engine right before the offender
    (the sequencer blocks on each in order — semantically identical)."""
    n = 0
    for func in nc.m.functions:
        for bb in func.blocks:
            out = []
            for inst in bb.instructions:
                si = inst.sync_info
                if si is not None and len(si.on_wait) > 1:
                    waits = list(si.on_wait)
                    for w in waits[:-1]:
                        n += 1
                        out.append(
                            mybir.InstEventSemaphore(
                                name=f"W-{inst.name}-{n}",
                                engine=inst.engine,
                                ins=[],
                                outs=[],
                                debug=inst.debug,
                                sync_info=mybir.SyncInfo(
                                    on_wait=[w], on_update=[]
                                ),
                            )
                        )
                    inst.sync_info = mybir.SyncInfo(
                        on_wait=[waits[-1]], on_update=list(si.on_update)
                    )
                out.append(inst)
            bb.instructions = out
    return n


def _groups(nt):
    """Tile groups: up to GSZ tiles share one PSUM region / reduce / sel-MM."""
    return [range(g, min(g + GSZ, nt)) for g in range(0, nt, GSZ)]


def _build_nc(nt, t_slots):
    import concourse.bass as bass
    import concourse.mybir as mybir
    import concourse.tile as tile
    from concourse.bass import ts

    bf16, f32 = mybir.dt.bfloat16, mybir.dt.float32
    ctile = BD_PER * t_slots
    nc = bass.Bass("TRN2", target_bir_lowering=False, debug=False)
    qryT = nc.dram_tensor("qryT", [K_EXT, nt * 128], bf16, kind="ExternalInput").ap()
    docT = nc.dram_tensor("docT", [K_EXT, nt * ctile], bf16, kind="ExternalInput").ap()
    selT = nc.dram_tensor("selT", [128, 8 * nt], f32, kind="ExternalInput").ap()
    out = nc.dram_tensor("out", [64, 16 * nt], f32, kind="ExternalOutput").ap()

    grps = _groups(nt)
    with tile.TileContext(nc) as tc:
        with (
            tc.tile_pool(name="inp", bufs=1) as inp,
            tc.tile_pool(name="psum", bufs=2, space="PSUM") as psum,
            tc.tile_pool(name="accp", bufs=1) as accp,
        ):
            # PE warm-up: junk matmuls during the DMA head so the HAM
            # clock-gate reaches 8/8 before the real work starts
            scratch = inp.tile([K_EXT, 512], bf16)
            nc.vector.memset(scratch[:], 0.0)
            wps = psum.tile([128, 512], f32, tag="score")
            for _ in range(WARMUP_MMS):
                nc.tensor.matmul(
                    wps[:], scratch[:, 0:128], scratch[:], start=True, stop=True
                )

            qry_sb = inp.tile([K_EXT, nt * 128], bf16)
            doc_sb = inp.tile([K_EXT, nt * ctile], bf16)
            sel_sb = inp.tile([128, 8 * nt], f32)
            # per group: doc chunk on the HWDGE (sync) queue, qry chunk on the
            # SWDGE (gpsimd) queue so they land in parallel, in group order
            for gi, grp in enumerate(grps):
                qts = list(grp)
                c0, c1 = qts[0] * ctile, (qts[-1] + 1) * ctile
                q0, q1 = qts[0] * 128, (qts[-1] + 1) * 128
                nc.sync.dma_start(doc_sb[:, c0:c1], docT[:, c0:c1])
                nc.gpsimd.dma_start(qry_sb[:, q0:q1], qryT[:, q0:q1])
            nc.gpsimd.dma_start(sel_sb[:], selT[:])

            accum = accp.tile([128, 16 * nt], f32)
            accv = accp.tile([128, 16 * nt], f32)
            negoff = accp.tile([128, 1], f32)
            nc.vector.memset(negoff[:], -OFF)

            for grp in grps:
                qts = list(grp)
                gn = len(qts)
                ps = psum.tile([128, ctile * gn], f32, tag="score")
                for j, qt in enumerate(qts):
                    nc.tensor.matmul(
                        ps[:, j * ctile : (j + 1) * ctile],
                        qry_sb[:, ts(qt, 128)],
                        doc_sb[:, qt * ctile : (qt + 1) * ctile],
                        start=True,
                        stop=True,
                    )
                # per-(doc,tile) max over the t_slots token slots, straight
                # from PSUM: [128, gn*16]
                nc.vector.reduce_max(
                    accum[:, qts[0] * 16 : (qts[-1] + 1) * 16],
                    ps[:].rearrange("p (c t) -> p c t", t=t_slots),
                    axis=mybir.AxisListType.X,
                )
            # tok = relu(v - OFF) in one ScalarE pass over all tiles
            nc.scalar.activation(
                accv[:],
                accum[:],
                mybir.ActivationFunctionType.Relu,
                bias=negoff[:],
            )
            # per-q partition sums: for each group of up to 8 tiles, one
            # matmul with the q-membership selector as the stationary operand;
            # only the [8,16] diagonal blocks are used (host slices them out)
            osb = accp.tile([64, 16 * nt], f32)
            nc.vector.memset(osb[:], 0.0)
            for g, grp in enumerate(grps):
                qts = list(grp)
                gn = len(qts)
                c0 = qts[0] * 16
                fin = psum.tile([8 * gn, 16 * gn], f32, tag="score")
                nc.tensor.matmul(
                    fin[:],
                    sel_sb[:, qts[0] * 8 : (qts[-1] + 1) * 8],
                    accv[:, c0 : c0 + 16 * gn],
                    start=True,
                    stop=True,
                )
                if g % 2 == 0:
                    nc.vector.tensor_copy(osb[0 : 8 * gn, c0 : c0 + 16 * gn], fin[:])
                else:
                    nc.scalar.copy(osb[0 : 8 * gn, c0 : c0 + 16 * gn], fin[:])
            nc.sync.dma_start(out[:], osb[:])
    _split_multi_waits(nc, mybir)
    return nc


def _get_nc(nt, t_slots):
    _patch_ldw_opt()
    key = (nt, t_slots, WARMUP_MMS)
    if key not in _CACHE:
        _CACHE[key] = _build_nc(nt, t_slots)
    return _CACHE[key]


def _qry_row_mask(qry_attention_mask):
    """[Bq, Sq] bool: rows that can contribute (attended, not CLS/SEP)."""
    mask = np.asarray(qry_attention_mask, np.int64).copy()
    sep = mask.sum(axis=1) - 1
    mask[np.arange(Bq), sep] = 0
    mask[:, 0] = 0
    return mask.astype(bool)


def _plan_core(qry_ids, qrow_ok, doc_ids, t_slots):
    """Greedy vocab-range tiling for one core's doc slab.

    Returns (tiles, qrows_sorted, qcnt) where tiles is a list of id-lists;
    each tile takes the next sum(qcnt[ids]) rows of qrows_sorted.
    """
    dids = doc_ids.reshape(-1)
    ddoc = np.repeat(np.arange(BD_PER), Sd)
    slab_ids = np.unique(dids)
    keep = qrow_ok & np.isin(qry_ids, slab_ids)
    qrows = np.nonzero(keep)[0]
    qcnt = np.bincount(qry_ids[qrows], minlength=1000)
    dcnt = np.zeros((1000, BD_PER), np.int64)
    np.add.at(dcnt, (dids, ddoc), 1)
    active = (qcnt > 0) & (dcnt.sum(1) > 0)
    tiles, cur, cur_q = [], [], 0
    cur_d = np.zeros(BD_PER, np.int64)

    def emit(ids, nrows):
        # a single id can exceed 128 q-rows: replicate the range across tiles
        # (each q-row still sees the full doc-token set for its id)
        while nrows > 128:
            tiles.append((ids, 128))
            nrows -= 128
        tiles.append((ids, nrows))

    for v in np.nonzero(active)[0]:
        nq, nd = qcnt[v], dcnt[v]
        if cur and (
            cur_q + nq > 128 or np.any(cur_d + nd > t_slots) or len(cur) >= MAXIDS
        ):
            emit(cur, cur_q)
            cur, cur_q, cur_d = [], 0, np.zeros(BD_PER, np.int64)
        cur.append(v)
        cur_q += nq
        cur_d += nd
    if cur:
        emit(cur, cur_q)
    order = np.argsort(qry_ids[qrows], kind="stable")
    return tiles, qrows[order], qcnt


def _prepare_in_maps(inputs):
    qry_reps = np.asarray(inputs["qry_reps"], np.float32).reshape(-1, D)
    qry_reps = _bf16(qry_reps).astype(np.float32)
    qry_ids = np.asarray(inputs["qry_input_ids"], np.int64).reshape(-1)
    doc_reps_all = np.asarray(inputs["doc_reps"], np.float32)
    doc_ids_all = np.asarray(inputs["doc_input_ids"], np.int64)
    qrow_ok = _qry_row_mask(inputs["qry_attention_mask"]).reshape(-1)
    qpos_q = np.repeat(np.arange(Bq), Sq)

    t_slots = T_SLOTS
    plans = []
    for core in range(NCORES):
        sl = slice(core * BD_PER, (core + 1) * BD_PER)
        # bump t_slots if a single id packs more tokens into one doc than fits
        while True:
            tiles, qrows_s, qcnt = _plan_core(
                qry_ids, qrow_ok, doc_ids_all[sl], t_slots
            )
            ok = True
            for ids, _nr in tiles:
                if len(ids) == 1:
                    dloc = doc_ids_all[sl].reshape(-1)
                    cnt = np.bincount(
                        np.repeat(np.arange(BD_PER), Sd)[dloc == ids[0]],
                        minlength=BD_PER,
                    )
                    if cnt.max() > t_slots:
                        t_slots = int(cnt.max())
                        ok = False
                        break
            if ok:
                break
            plans = []
        plans.append((tiles, qrows_s, qcnt))
    nt = max(len(p[0]) for p in plans)
    ctile = BD_PER * t_slots

    in_maps = []
    for core in range(NCORES):
        sl = slice(core * BD_PER, (core + 1) * BD_PER)
        tiles, qrows_s, qcnt = plans[core]
        dids = doc_ids_all[sl].reshape(-1)
        ddoc = np.repeat(np.arange(BD_PER), Sd)
        dreps = _bf16(doc_reps_all[sl].reshape(-1, D).astype(np.float32)).astype(
            np.float32
        )
        qT = np.zeros((K_EXT, nt * 128), dtype=np.float32)
        dT = np.zeros((K_EXT, nt * ctile), dtype=np.float32)
        sel = np.zeros((128, 8 * nt), dtype=np.float32)
        ptr = 0
        for ti, (ids, n_in) in enumerate(tiles):
            idmap = {v: i for i, v in enumerate(ids)}
            rows = qrows_s[ptr : ptr + n_in]
            ptr += n_in
            lq = [idmap[v] for v in qry_ids[rows]]
            qT[:D, ti * 128 : ti * 128 + n_in] = qry_reps[rows].T
            qT[D:, ti * 128 : ti * 128 + n_in] = (ALPHA * _onehot6(lq)).T
            sel[np.arange(n_in), ti * 8 + qpos_q[rows]] = 1.0
            tsel = np.nonzero(np.isin(dids, ids))[0]
            for d in range(BD_PER):
                dtok = tsel[ddoc[tsel] == d]
                m = len(dtok)
                c0 = ti * ctile + d * t_slots
                dT[:D, c0 : c0 + m] = dreps[dtok].T
                dT[D:, c0 : c0 + m] = (ALPHA * _onehot6([idmap[v] for v in dids[dtok]])).T
        in_maps.append(
            {"qryT": _bf16(qT), "docT": _bf16(dT), "selT": sel}
        )
    return in_maps, nt, t_slots


def _assemble(inputs, results, nt):
    toks = np.zeros((Bq, Bd), dtype=np.float32)
    for core in range(NCORES):
        osb = np.asarray(results[core]["out"], np.float32)  # [64, 16*nt]
        part = np.zeros((Bq, BD_PER), dtype=np.float32)
        for g, grp in enumerate(_groups(nt)):
            for tl, qt in enumerate(grp):
                part += osb[8 * tl : 8 * tl + 8, qt * 16 : (qt + 1) * 16]
        toks[:, core * BD_PER : (core + 1) * BD_PER] = part
    cls = np.asarray(inputs["qry_cls"], np.float32) @ np.asarray(
        inputs["doc_cls"], np.float32
    ).T
    scores = toks + cls
    return scores.max(axis=0).reshape(-1).astype(np.float32)


def _ensure_ntff_hook():
    """This container's antenv lacks axon_hooks; synthesize the module and
    register the ctypes-based NTFF profile hook so trace=True works."""
    import sys
    import types

    if "antenv.axon_hooks" in sys.modules:
        return
    mod = types.ModuleType("antenv.axon_hooks")
    state = {"hook": None}
    mod.set_axon_ntff_profile_hook = lambda h: state.__setitem__("hook", h)
    mod.get_axon_ntff_profile_hook = lambda: state["hook"]
    sys.modules["antenv.axon_hooks"] = mod
    try:
        import antenv

        antenv.axon_hooks = mod
    except ImportError:
        pass
    try:
        from trn_agent_boot.trn_boot import _ntff_profile_via_ctypes

        mod.set_axon_ntff_profile_hook(
            _ntff_profile_via_ctypes("/opt/axon/libaxon_pjrt.so")
        )
    except Exception:
        pass


def run(inputs, trace=False, **kwargs):
    """Run on the 8 NeuronCores; returns (output, BassKernelResults)."""
    from concourse.bass_utils import run_bass_kernel_spmd

    if trace:
        _ensure_ntff_hook()
    in_maps, nt, t_slots = _prepare_in_maps(inputs)
    nc = _get_nc(nt, t_slots)
    res = run_bass_kernel_spmd(
        nc, in_maps, core_ids=list(range(NCORES)), trace=trace, **kwargs
    )
    return _assemble(inputs, res.results, nt), res


def kernel(**inputs) -> np.ndarray:
    out, _ = run(inputs)
    return out


# revision 9
# speedup vs baseline: 2.7395x; 1.1559x over previous
"""COIL sparse-attention scoring kernel for 8 Trainium2 NeuronCores.

Strategy: vocab-range-blocked sparse scoring
--------------------------------------------
Shard the doc axis (Bd=128) across the 8 cores (16 docs each); qry tensors are
replicated. Only (q-token, doc-token) pairs with EQUAL ids contribute, so the
full [128 q, 2048 doc-token] cartesian per q-tile is 8x wasteful. Instead the
host sorts q-rows by token id and greedily packs contiguous vocab ranges into
tiles: each tile holds <=128 q-rows whose ids fall in a range of <=36 distinct
ids, and only the doc tokens with ids in that range, grouped per doc into
T=16 slots -> 16 docs x 16 slots = 256 columns per tile (vs 2048).

Exact-match detection stays folded into the matmul: each id is encoded by its
LOCAL index within the range as a 2-digit base-6 one-hot scaled by ALPHA=32 and
appended to the bf16 reps (K = 32 + 12 = 44). Then

    v[q, col] = S[q, col] + 1024 * match_digits,  match_digits == 2 iff equal,

so tok[q, doc] = relu(max_slots v - 2048) reproduces the reference masked max
(non-match v <= 1024 + |S|max ~ 1064, match v >= 2048 - |S|max ~ 2008).

Per core: ~29 tiles; one [44,128]x[44,256] matmul per tile into a shared PSUM
region of 8 tiles (4 banks); ONE DVE reduce_max (t=16) per 8-tile group
produces the per-doc maxes; a single ScalarE relu(x-OFF) pass converts them to
tok scores; per-q sums are ones/selector matmuls on the PE (partition-dim sum).
CLS scores and the final max over the 8 query chunks are done on host (tiny).
"""

import os
import numpy as np
import ml_dtypes

Bq, Sq, Bd, Sd, D, Dc = 8, 512, 128, 128, 32, 768
NCORES = 8
BD_PER = Bd // NCORES          # 16 docs per core
K_EXT = 44                     # 32 rep dims + 12 one-hot dims (2 digits base 6)
T_SLOTS = int(os.environ.get("KERNEL_T_SLOTS", "8"))  # doc slots per (doc, range)
MAXIDS = 36                    # max distinct ids per range (6*6 one-hot codes)
GSZ = 8                        # tiles per reduce/sel chunk (PSUM region = 1 chunk)
ALPHA = 32.0
OFF = 2.0 * ALPHA * ALPHA      # 2048: offset of a full 2-digit match
WARMUP_MMS = int(os.environ.get("KERNEL_WARMUP_MMS", "4"))

_CACHE = {}


def _bf16(x):
    return x.astype(ml_dtypes.bfloat16)


def _onehot6(idx):
    """local indices [N] in [0,36) -> [N,12] 2-digit base-6 one-hot (f32)."""
    idx = np.asarray(idx, np.int64)
    H = np.zeros((len(idx), 12), dtype=np.float32)
    r = np.arange(len(idx))
    H[r, idx % 6] = 1.0
    H[r, 6 + idx // 6] = 1.0
    return H


_LDW_PATCHED = False


def _patch_ldw_opt():
    """bir_verify_and_optimise hardcodes --enable-ldw-opt=false; opt-in knob."""
    global _LDW_PATCHED
    if _LDW_PATCHED or not os.environ.get("KERNEL_LDW_OPT"):
        return
    import concourse.bass_utils as bu

    orig = bu.get_walrus_args

    def patched(*a, **k):
        return orig(*a, **k) + ["--enable-ldw-opt=true"]

    bu.get_walrus_args = patched
    _LDW_PATCHED = True


def _split_multi_waits(nc, mybir):
    """This container's walrus accepts only ONE sync-wait per instruction
    ("Too many sync wait commands"). Hoist extra waits into standalone
    EventSemaphore instructions on the same engine right before the offender
    (the sequencer blocks on each in order — semantically identical)."""
    n = 0
    for func in nc.m.functions:
        for bb in func.blocks:
            out = []
            for inst in bb.instructions:
                si = inst.sync_info
                if si is not None and len(si.on_wait) > 1:
                    waits = list(si.on_wait)
                    for w in waits[:-1]:
                        n += 1
                        out.append(
                            mybir.InstEventSemaphore(
                                name=f"W-{inst.name}-{n}",
                                engine=inst.engine,
                                ins=[],
                                outs=[],
                                debug=inst.debug,
                                sync_info=mybir.SyncInfo(
                                    on_wait=[w], on_update=[]
                                ),
                            )
                        )
                    inst.sync_info = mybir.SyncInfo(
                        on_wait=[waits[-1]], on_update=list(si.on_update)
                    )
                out.append(inst)
            bb.instructions = out
    return n


def _groups(nt):
    """Tile groups: up to GSZ tiles share one PSUM region / reduce / sel-MM."""
    return [range(g, min(g + GSZ, nt)) for g in range(0, nt, GSZ)]


def _build_nc(nt, t_slots):
    import concourse.bass as bass
    import concourse.mybir as mybir
    import concourse.tile as tile
    from concourse.bass import ts

    bf16, f32 = mybir.dt.bfloat16, mybir.dt.float32
    ctile = BD_PER * t_slots
    nc = bass.Bass("TRN2", target_bir_lowering=False, debug=False)
    qryT = nc.dram_tensor("qryT", [K_EXT, nt * 128], bf16, kind="ExternalInput").ap()
    docT = nc.dram_tensor("docT", [K_EXT, nt * ctile], bf16, kind="ExternalInput").ap()
    selT = nc.dram_tensor("selT", [128, 8 * nt], f32, kind="ExternalInput").ap()
    out = nc.dram_tensor("out", [64, 16 * nt], f32, kind="ExternalOutput").ap()

    grps = _groups(nt)
    with tile.TileContext(nc) as tc:
        with (
            tc.tile_pool(name="inp", bufs=1) as inp,
            tc.tile_pool(name="psum", bufs=3, space="PSUM") as psum,
            tc.tile_pool(name="accp", bufs=1) as accp,
        ):
            # PE warm-up: junk matmuls during the DMA head so the HAM
            # clock-gate reaches 8/8 before the real work starts
            scratch = inp.tile([K_EXT, 512], bf16)
            nc.vector.memset(scratch[:], 0.0)
            wps = psum.tile([128, 512], f32, tag="score")
            for _ in range(WARMUP_MMS):
                nc.tensor.matmul(
                    wps[:], scratch[:, 0:128], scratch[:], start=True, stop=True
                )

            qry_sb = inp.tile([K_EXT, nt * 128], bf16)
            doc_sb = inp.tile([K_EXT, nt * ctile], bf16)
            sel_sb = inp.tile([128, 8 * nt], f32)
            # two chunks each, split across the HWDGE (sync) and SWDGE
            # (gpsimd) queues; sel is only needed once the first chunk's
            # reduce+relu lands, so it loads third
            mid = max(1, nt // 2)
            nc.sync.dma_start(doc_sb[:, : mid * ctile], docT[:, : mid * ctile])
            nc.gpsimd.dma_start(qry_sb[:, : mid * 128], qryT[:, : mid * 128])
            nc.sync.dma_start(doc_sb[:, mid * ctile :], docT[:, mid * ctile :])
            nc.gpsimd.dma_start(sel_sb[:], selT[:])
            nc.gpsimd.dma_start(qry_sb[:, mid * 128 :], qryT[:, mid * 128 :])

            accum = accp.tile([128, 16 * nt], f32)
            accv = accp.tile([128, 16 * nt], f32)
            negoff = accp.tile([128, 1], f32)
            nc.vector.memset(negoff[:], -OFF)
            osb = accp.tile([64, 16 * nt], f32)
            nc.vector.memset(osb[:], 0.0)

            # per chunk of GSZ tiles: MMs -> DVE reduce -> ScalarE relu ->
            # selector MM.  The selector MM for chunk r is issued AFTER chunk
            # r+1's MMs so the PE never stalls waiting for relu(r).
            def sel_mm(g, grp):
                qts = list(grp)
                gn = len(qts)
                c0 = qts[0] * 16
                fin = psum.tile([8 * gn, 16 * gn], f32, tag="score")
                nc.tensor.matmul(
                    fin[:],
                    sel_sb[:, qts[0] * 8 : (qts[-1] + 1) * 8],
                    accv[:, c0 : c0 + 16 * gn],
                    start=True,
                    stop=True,
                )
                if g % 2 == 0:
                    nc.vector.tensor_copy(osb[0 : 8 * gn, c0 : c0 + 16 * gn], fin[:])
                else:
                    nc.scalar.copy(osb[0 : 8 * gn, c0 : c0 + 16 * gn], fin[:])

            for g, grp in enumerate(grps):
                qts = list(grp)
                gn = len(qts)
                ps = psum.tile([128, ctile * gn], f32, tag="score")
                for j, qt in enumerate(qts):
                    nc.tensor.matmul(
                        ps[:, j * ctile : (j + 1) * ctile],
                        qry_sb[:, ts(qt, 128)],
                        doc_sb[:, qt * ctile : (qt + 1) * ctile],
                        start=True,
                        stop=True,
                    )
                if g > 0:
                    sel_mm(g - 1, grps[g - 1])
                # per-(doc,tile) max over the t_slots token slots, straight
                # from PSUM: [128, gn*16]
                c0 = qts[0] * 16
                nc.vector.reduce_max(
                    accum[:, c0 : (qts[-1] + 1) * 16],
                    ps[:].rearrange("p (c t) -> p c t", t=t_slots),
                    axis=mybir.AxisListType.X,
                )
                # tok = relu(v - OFF) for this chunk on ScalarE
                nc.scalar.activation(
                    accv[:, c0 : (qts[-1] + 1) * 16],
                    accum[:, c0 : (qts[-1] + 1) * 16],
                    mybir.ActivationFunctionType.Relu,
                    bias=negoff[:],
                )
            sel_mm(len(grps) - 1, grps[-1])
            nc.sync.dma_start(out[:], osb[:])
    _split_multi_waits(nc, mybir)
    return nc


def _get_nc(nt, t_slots):
    _patch_ldw_opt()
    key = (nt, t_slots, WARMUP_MMS)
    if key not in _CACHE:
        _CACHE[key] = _build_nc(nt, t_slots)
    return _CACHE[key]


def _qry_row_mask(qry_attention_mask):
    """[Bq, Sq] bool: rows that can contribute (attended, not CLS/SEP)."""
    mask = np.asarray(qry_attention_mask, np.int64).copy()
    sep = mask.sum(axis=1) - 1
    mask[np.arange(Bq), sep] = 0
    mask[:, 0] = 0
    return mask.astype(bool)


def _plan_core(qry_ids, qrow_ok, doc_ids, t_slots):
    """Greedy vocab-range tiling for one core's doc slab.

    Returns (tiles, qrows_sorted, qcnt) where tiles is a list of id-lists;
    each tile takes the next sum(qcnt[ids]) rows of qrows_sorted.
    """
    dids = doc_ids.reshape(-1)
    ddoc = np.repeat(np.arange(BD_PER), Sd)
    slab_ids = np.unique(dids)
    keep = qrow_ok & np.isin(qry_ids, slab_ids)
    qrows = np.nonzero(keep)[0]
    qcnt = np.bincount(qry_ids[qrows], minlength=1000)
    dcnt = np.zeros((1000, BD_PER), np.int64)
    np.add.at(dcnt, (dids, ddoc), 1)
    active = (qcnt > 0) & (dcnt.sum(1) > 0)
    tiles, cur, cur_q = [], [], 0
    cur_d = np.zeros(BD_PER, np.int64)

    def emit(ids, nrows):
        # a single id can exceed 128 q-rows: replicate the range across tiles
        # (each q-row still sees the full doc-token set for its id)
        while nrows > 128:
            tiles.append((ids, 128))
            nrows -= 128
        tiles.append((ids, nrows))

    for v in np.nonzero(active)[0]:
        nq, nd = qcnt[v], dcnt[v]
        if cur and (
            cur_q + nq > 128 or np.any(cur_d + nd > t_slots) or len(cur) >= MAXIDS
        ):
            emit(cur, cur_q)
            cur, cur_q, cur_d = [], 0, np.zeros(BD_PER, np.int64)
        cur.append(v)
        cur_q += nq
        cur_d += nd
    if cur:
        emit(cur, cur_q)
    order = np.argsort(qry_ids[qrows], kind="stable")
    return tiles, qrows[order], qcnt


def _prepare_in_maps(inputs):
    qry_reps = np.asarray(inputs["qry_reps"], np.float32).reshape(-1, D)
    qry_reps = _bf16(qry_reps).astype(np.float32)
    qry_ids = np.asarray(inputs["qry_input_ids"], np.int64).reshape(-1)
    doc_reps_all = np.asarray(inputs["doc_reps"], np.float32)
    doc_ids_all = np.asarray(inputs["doc_input_ids"], np.int64)
    qrow_ok = _qry_row_mask(inputs["qry_attention_mask"]).reshape(-1)
    qpos_q = np.repeat(np.arange(Bq), Sq)

    t_slots = T_SLOTS
    plans = []
    for core in range(NCORES):
        sl = slice(core * BD_PER, (core + 1) * BD_PER)
        # bump t_slots if a single id packs more tokens into one doc than fits
        while True:
            tiles, qrows_s, qcnt = _plan_core(
                qry_ids, qrow_ok, doc_ids_all[sl], t_slots
            )
            ok = True
            for ids, _nr in tiles:
                if len(ids) == 1:
                    dloc = doc_ids_all[sl].reshape(-1)
                    cnt = np.bincount(
                        np.repeat(np.arange(BD_PER), Sd)[dloc == ids[0]],
                        minlength=BD_PER,
                    )
                    if cnt.max() > t_slots:
                        t_slots = int(cnt.max())
                        ok = False
                        break
            if ok:
                break
            plans = []
        plans.append((tiles, qrows_s, qcnt))
    nt = max(len(p[0]) for p in plans)
    ctile = BD_PER * t_slots

    in_maps = []
    for core in range(NCORES):
        sl = slice(core * BD_PER, (core + 1) * BD_PER)
        tiles, qrows_s, qcnt = plans[core]
        dids = doc_ids_all[sl].reshape(-1)
        ddoc = np.repeat(np.arange(BD_PER), Sd)
        dreps = _bf16(doc_reps_all[sl].reshape(-1, D).astype(np.float32)).astype(
            np.float32
        )
        qT = np.zeros((K_EXT, nt * 128), dtype=np.float32)
        dT = np.zeros((K_EXT, nt * ctile), dtype=np.float32)
        sel = np.zeros((128, 8 * nt), dtype=np.float32)
        ptr = 0
        for ti, (ids, n_in) in enumerate(tiles):
            idmap = {v: i for i, v in enumerate(ids)}
            rows = qrows_s[ptr : ptr + n_in]
            ptr += n_in
            lq = [idmap[v] for v in qry_ids[rows]]
            qT[:D, ti * 128 : ti * 128 + n_in] = qry_reps[rows].T
            qT[D:, ti * 128 : ti * 128 + n_in] = (ALPHA * _onehot6(lq)).T
            sel[np.arange(n_in), ti * 8 + qpos_q[rows]] = 1.0
            tsel = np.nonzero(np.isin(dids, ids))[0]
            for d in range(BD_PER):
                dtok = tsel[ddoc[tsel] == d]
                m = len(dtok)
                c0 = ti * ctile + d * t_slots
                dT[:D, c0 : c0 + m] = dreps[dtok].T
                dT[D:, c0 : c0 + m] = (ALPHA * _onehot6([idmap[v] for v in dids[dtok]])).T
        in_maps.append(
            {"qryT": _bf16(qT), "docT": _bf16(dT), "selT": sel}
        )
    return in_maps, nt, t_slots


def _assemble(inputs, results, nt):
    toks = np.zeros((Bq, Bd), dtype=np.float32)
    for core in range(NCORES):
        osb = np.asarray(results[core]["out"], np.float32)  # [64, 16*nt]
        part = np.zeros((Bq, BD_PER), dtype=np.float32)
        for g, grp in enumerate(_groups(nt)):
            for tl, qt in enumerate(grp):
                part += osb[8 * tl : 8 * tl + 8, qt * 16 : (qt + 1) * 16]
        toks[:, core * BD_PER : (core + 1) * BD_PER] = part
    cls = np.asarray(inputs["qry_cls"], np.float32) @ np.asarray(
        inputs["doc_cls"], np.float32
    ).T
    scores = toks + cls
    return scores.max(axis=0).reshape(-1).astype(np.float32)


def _ensure_ntff_hook():
    """This container's antenv lacks axon_hooks; synthesize the module and
    register the ctypes-based NTFF profile hook so trace=True works."""
    import sys
    import types

    if "antenv.axon_hooks" in sys.modules:
        return
    mod = types.ModuleType("antenv.axon_hooks")
    state = {"hook": None}
    mod.set_axon_ntff_profile_hook = lambda h: state.__setitem__("hook", h)
    mod.get_axon_ntff_profile_hook = lambda: state["hook"]
    sys.modules["antenv.axon_hooks"] = mod
    try:
        import antenv

        antenv.axon_hooks = mod
    except ImportError:
        pass
    try:
        from trn_agent_boot.trn_boot import _ntff_profile_via_ctypes

        mod.set_axon_ntff_profile_hook(
            _ntff_profile_via_ctypes("/opt/axon/libaxon_pjrt.so")
        )
    except Exception:
        pass


def run(inputs, trace=False, **kwargs):
    """Run on the 8 NeuronCores; returns (output, BassKernelResults)."""
    from concourse.bass_utils import run_bass_kernel_spmd

    if trace:
        _ensure_ntff_hook()
    in_maps, nt, t_slots = _prepare_in_maps(inputs)
    nc = _get_nc(nt, t_slots)
    res = run_bass_kernel_spmd(
        nc, in_maps, core_ids=list(range(NCORES)), trace=trace, **kwargs
    )
    return _assemble(inputs, res.results, nt), res


def kernel(**inputs) -> np.ndarray:
    out, _ = run(inputs)
    return out


# revision 14
# speedup vs baseline: 2.9683x; 1.0835x over previous
"""COIL sparse-attention scoring kernel for 8 Trainium2 NeuronCores.

Strategy: vocab-range-blocked sparse scoring
--------------------------------------------
Shard the doc axis (Bd=128) across the 8 cores (16 docs each); qry tensors are
replicated. Only (q-token, doc-token) pairs with EQUAL ids contribute, so the
full [128 q, 2048 doc-token] cartesian per q-tile is 8x wasteful. Instead the
host sorts q-rows by token id and greedily packs contiguous vocab ranges into
tiles: each tile holds <=128 q-rows whose ids fall in a range of <=36 distinct
ids, and only the doc tokens with ids in that range, grouped per doc into
T=16 slots -> 16 docs x 16 slots = 256 columns per tile (vs 2048).

Exact-match detection stays folded into the matmul: each id is encoded by its
LOCAL index within the range as a 2-digit base-6 one-hot scaled by ALPHA=32 and
appended to the bf16 reps (K = 32 + 12 = 44). Then

    v[q, col] = S[q, col] + 1024 * match_digits,  match_digits == 2 iff equal,

so tok[q, doc] = relu(max_slots v - 2048) reproduces the reference masked max
(non-match v <= 1024 + |S|max ~ 1064, match v >= 2048 - |S|max ~ 2008).

Per core: ~29 tiles; one [44,128]x[44,256] matmul per tile into a shared PSUM
region of 8 tiles (4 banks); ONE DVE reduce_max (t=16) per 8-tile group
produces the per-doc maxes; a single ScalarE relu(x-OFF) pass converts them to
tok scores; per-q sums are ones/selector matmuls on the PE (partition-dim sum).
CLS scores and the final max over the 8 query chunks are done on host (tiny).
"""

import os
import numpy as np
import ml_dtypes

Bq, Sq, Bd, Sd, D, Dc = 8, 512, 128, 128, 32, 768
NCORES = 8
BD_PER = Bd // NCORES          # 16 docs per core
K_EXT = 44                     # 32 rep dims + 12 one-hot dims (2 digits base 6)
T_SLOTS = int(os.environ.get("KERNEL_T_SLOTS", "8"))  # doc slots per (doc, range)
MAXIDS = 36                    # max distinct ids per range (6*6 one-hot codes)
GSZ = 8                        # tiles per reduce/sel chunk (PSUM region = 1 chunk)
ALPHA = 32.0
OFF = 2.0 * ALPHA * ALPHA      # 2048: offset of a full 2-digit match
WARMUP_MMS = int(os.environ.get("KERNEL_WARMUP_MMS", "0"))
SEL_LOOKAHEAD = int(os.environ.get("KERNEL_SEL_LOOKAHEAD", "2"))

_CACHE = {}


def _bf16(x):
    return x.astype(ml_dtypes.bfloat16)


def _onehot6(idx):
    """local indices [N] in [0,36) -> [N,12] 2-digit base-6 one-hot (f32)."""
    idx = np.asarray(idx, np.int64)
    H = np.zeros((len(idx), 12), dtype=np.float32)
    r = np.arange(len(idx))
    H[r, idx % 6] = 1.0
    H[r, 6 + idx // 6] = 1.0
    return H


_LDW_PATCHED = False


def _patch_ldw_opt():
    """bir_verify_and_optimise hardcodes --enable-ldw-opt=false; opt-in knob."""
    global _LDW_PATCHED
    if _LDW_PATCHED or not os.environ.get("KERNEL_LDW_OPT"):
        return
    import concourse.bass_utils as bu

    orig = bu.get_walrus_args

    def patched(*a, **k):
        return orig(*a, **k) + ["--enable-ldw-opt=true"]

    bu.get_walrus_args = patched
    _LDW_PATCHED = True


def _split_multi_waits(nc, mybir):
    """This container's walrus accepts only ONE sync-wait per instruction
    ("Too many sync wait commands"). Hoist extra waits into standalone
    EventSemaphore instructions on the same engine right before the offender
    (the sequencer blocks on each in order — semantically identical)."""
    n = 0
    for func in nc.m.functions:
        for bb in func.blocks:
            out = []
            for inst in bb.instructions:
                si = inst.sync_info
                if si is not None and len(si.on_wait) > 1:
                    waits = list(si.on_wait)
                    for w in waits[:-1]:
                        n += 1
                        out.append(
                            mybir.InstEventSemaphore(
                                name=f"W-{inst.name}-{n}",
                                engine=inst.engine,
                                ins=[],
                                outs=[],
                                debug=inst.debug,
                                sync_info=mybir.SyncInfo(
                                    on_wait=[w], on_update=[]
                                ),
                            )
                        )
                    inst.sync_info = mybir.SyncInfo(
                        on_wait=[waits[-1]], on_update=list(si.on_update)
                    )
                out.append(inst)
            bb.instructions = out
    return n


def _groups(nt):
    """Tile groups: up to GSZ tiles share one PSUM region / reduce / sel-MM."""
    return [range(g, min(g + GSZ, nt)) for g in range(0, nt, GSZ)]


def _build_nc(nt, t_slots):
    import concourse.bass as bass
    import concourse.mybir as mybir
    import concourse.tile as tile
    from concourse.bass import ts

    bf16, f32 = mybir.dt.bfloat16, mybir.dt.float32
    ctile = BD_PER * t_slots
    nc = bass.Bass("TRN2", target_bir_lowering=False, debug=False)
    qryT = nc.dram_tensor("qryT", [K_EXT, nt * 128], bf16, kind="ExternalInput").ap()
    docT = nc.dram_tensor("docT", [K_EXT, nt * ctile], bf16, kind="ExternalInput").ap()
    selT = nc.dram_tensor("selT", [128, 8 * nt], f32, kind="ExternalInput").ap()
    out = nc.dram_tensor("out", [64, 16 * nt], f32, kind="ExternalOutput").ap()

    grps = _groups(nt)
    with tile.TileContext(nc) as tc:
        with (
            tc.tile_pool(name="inp", bufs=1) as inp,
            tc.tile_pool(name="psum", bufs=3, space="PSUM") as psum,
            tc.tile_pool(name="accp", bufs=1) as accp,
        ):
            qry_sb = inp.tile([K_EXT, nt * 128], bf16)
            doc_sb = inp.tile([K_EXT, nt * ctile], bf16)
            sel_sb = inp.tile([128, 8 * nt], f32)
            # two chunks each; doc on the SP HWDGE queue-trigger, qry on the
            # Activation one so the (~1us each) triggers process in parallel.
            # GpSimd/SWDGE is avoided entirely: its end-of-kernel dge_drain is
            # expensive.  sel is only needed once the first chunk's
            # reduce+relu lands, so it loads third.
            mid = max(1, nt // 2)
            nc.sync.dma_start(doc_sb[:, : mid * ctile], docT[:, : mid * ctile])
            nc.scalar.dma_start(qry_sb[:, : mid * 128], qryT[:, : mid * 128])
            nc.sync.dma_start(doc_sb[:, mid * ctile :], docT[:, mid * ctile :])
            nc.scalar.dma_start(qry_sb[:, mid * 128 :], qryT[:, mid * 128 :])
            nc.sync.dma_start(sel_sb[:], selT[:])

            # PE warm-up: junk matmuls during the DMA head keep the HAM
            # clock from gating down before the real work starts
            if WARMUP_MMS:
                scratch = inp.tile([K_EXT, 512], bf16)
                nc.vector.memset(scratch[:], 0.0)
                wps = psum.tile([128, 512], f32, tag="score")
                for _ in range(WARMUP_MMS):
                    nc.tensor.matmul(
                        wps[:], scratch[:, 0:128], scratch[:], start=True, stop=True
                    )

            accum = accp.tile([128, 16 * nt], f32)
            accv = accp.tile([128, 16 * nt], f32)
            negoff = accp.tile([128, 1], f32)
            nc.vector.memset(negoff[:], -OFF)
            osb = accp.tile([64, 16 * nt], f32)
            nc.vector.memset(osb[:], 0.0)

            # per chunk of GSZ tiles: MMs -> DVE reduce -> ScalarE relu ->
            # selector MM.  The selector MM for chunk r is issued AFTER chunk
            # r+1's MMs so the PE never stalls waiting for relu(r).
            def sel_mm(g, grp):
                qts = list(grp)
                gn = len(qts)
                c0 = qts[0] * 16
                fin = psum.tile([8 * gn, 16 * gn], f32, tag="score")
                nc.tensor.matmul(
                    fin[:],
                    sel_sb[:, qts[0] * 8 : (qts[-1] + 1) * 8],
                    accv[:, c0 : c0 + 16 * gn],
                    start=True,
                    stop=True,
                )
                if g % 2 == 0:
                    nc.vector.tensor_copy(osb[0 : 8 * gn, c0 : c0 + 16 * gn], fin[:])
                else:
                    nc.scalar.copy(osb[0 : 8 * gn, c0 : c0 + 16 * gn], fin[:])

            for g, grp in enumerate(grps):
                qts = list(grp)
                gn = len(qts)
                ps = psum.tile([128, ctile * gn], f32, tag="score")
                for j, qt in enumerate(qts):
                    nc.tensor.matmul(
                        ps[:, j * ctile : (j + 1) * ctile],
                        qry_sb[:, ts(qt, 128)],
                        doc_sb[:, qt * ctile : (qt + 1) * ctile],
                        start=True,
                        stop=True,
                    )
                if g >= SEL_LOOKAHEAD:
                    sel_mm(g - SEL_LOOKAHEAD, grps[g - SEL_LOOKAHEAD])
                # per-(doc,tile) max over the t_slots token slots, straight
                # from PSUM: [128, gn*16]
                c0 = qts[0] * 16
                nc.vector.reduce_max(
                    accum[:, c0 : (qts[-1] + 1) * 16],
                    ps[:].rearrange("p (c t) -> p c t", t=t_slots),
                    axis=mybir.AxisListType.X,
                )
                # tok = relu(v - OFF) for this chunk on ScalarE
                nc.scalar.activation(
                    accv[:, c0 : (qts[-1] + 1) * 16],
                    accum[:, c0 : (qts[-1] + 1) * 16],
                    mybir.ActivationFunctionType.Relu,
                    bias=negoff[:],
                )
            for g in range(max(0, len(grps) - SEL_LOOKAHEAD), len(grps)):
                sel_mm(g, grps[g])
            nc.sync.dma_start(out[:], osb[:])
    _split_multi_waits(nc, mybir)
    return nc


def _get_nc(nt, t_slots):
    _patch_ldw_opt()
    key = (nt, t_slots, WARMUP_MMS, SEL_LOOKAHEAD)
    if key not in _CACHE:
        _CACHE[key] = _build_nc(nt, t_slots)
    return _CACHE[key]


def _qry_row_mask(qry_attention_mask):
    """[Bq, Sq] bool: rows that can contribute (attended, not CLS/SEP)."""
    mask = np.asarray(qry_attention_mask, np.int64).copy()
    sep = mask.sum(axis=1) - 1
    mask[np.arange(Bq), sep] = 0
    mask[:, 0] = 0
    return mask.astype(bool)


def _plan_core(qry_ids, qrow_ok, doc_ids, t_slots):
    """Greedy vocab-range tiling for one core's doc slab.

    Returns (tiles, qrows_sorted, qcnt) where tiles is a list of id-lists;
    each tile takes the next sum(qcnt[ids]) rows of qrows_sorted.
    """
    dids = doc_ids.reshape(-1)
    ddoc = np.repeat(np.arange(BD_PER), Sd)
    slab_ids = np.unique(dids)
    keep = qrow_ok & np.isin(qry_ids, slab_ids)
    qrows = np.nonzero(keep)[0]
    qcnt = np.bincount(qry_ids[qrows], minlength=1000)
    dcnt = np.zeros((1000, BD_PER), np.int64)
    np.add.at(dcnt, (dids, ddoc), 1)
    active = (qcnt > 0) & (dcnt.sum(1) > 0)
    tiles, cur, cur_q = [], [], 0
    cur_d = np.zeros(BD_PER, np.int64)

    def emit(ids, nrows):
        # a single id can exceed 128 q-rows: replicate the range across tiles
        # (each q-row still sees the full doc-token set for its id)
        while nrows > 128:
            tiles.append((ids, 128))
            nrows -= 128
        tiles.append((ids, nrows))

    for v in np.nonzero(active)[0]:
        nq, nd = qcnt[v], dcnt[v]
        if cur and (
            cur_q + nq > 128 or np.any(cur_d + nd > t_slots) or len(cur) >= MAXIDS
        ):
            emit(cur, cur_q)
            cur, cur_q, cur_d = [], 0, np.zeros(BD_PER, np.int64)
        cur.append(v)
        cur_q += nq
        cur_d += nd
    if cur:
        emit(cur, cur_q)
    order = np.argsort(qry_ids[qrows], kind="stable")
    return tiles, qrows[order], qcnt


def _prepare_in_maps(inputs):
    qry_reps = np.asarray(inputs["qry_reps"], np.float32).reshape(-1, D)
    qry_reps = _bf16(qry_reps).astype(np.float32)
    qry_ids = np.asarray(inputs["qry_input_ids"], np.int64).reshape(-1)
    doc_reps_all = np.asarray(inputs["doc_reps"], np.float32)
    doc_ids_all = np.asarray(inputs["doc_input_ids"], np.int64)
    qrow_ok = _qry_row_mask(inputs["qry_attention_mask"]).reshape(-1)
    qpos_q = np.repeat(np.arange(Bq), Sq)

    t_slots = T_SLOTS
    plans = []
    for core in range(NCORES):
        sl = slice(core * BD_PER, (core + 1) * BD_PER)
        # bump t_slots if a single id packs more tokens into one doc than fits
        while True:
            tiles, qrows_s, qcnt = _plan_core(
                qry_ids, qrow_ok, doc_ids_all[sl], t_slots
            )
            ok = True
            for ids, _nr in tiles:
                if len(ids) == 1:
                    dloc = doc_ids_all[sl].reshape(-1)
                    cnt = np.bincount(
                        np.repeat(np.arange(BD_PER), Sd)[dloc == ids[0]],
                        minlength=BD_PER,
                    )
                    if cnt.max() > t_slots:
                        t_slots = int(cnt.max())
                        ok = False
                        break
            if ok:
                break
            plans = []
        plans.append((tiles, qrows_s, qcnt))
    nt = max(len(p[0]) for p in plans)
    ctile = BD_PER * t_slots

    in_maps = []
    for core in range(NCORES):
        sl = slice(core * BD_PER, (core + 1) * BD_PER)
        tiles, qrows_s, qcnt = plans[core]
        dids = doc_ids_all[sl].reshape(-1)
        ddoc = np.repeat(np.arange(BD_PER), Sd)
        dreps = _bf16(doc_reps_all[sl].reshape(-1, D).astype(np.float32)).astype(
            np.float32
        )
        qT = np.zeros((K_EXT, nt * 128), dtype=np.float32)
        dT = np.zeros((K_EXT, nt * ctile), dtype=np.float32)
        sel = np.zeros((128, 8 * nt), dtype=np.float32)
        ptr = 0
        for ti, (ids, n_in) in enumerate(tiles):
            idmap = {v: i for i, v in enumerate(ids)}
            rows = qrows_s[ptr : ptr + n_in]
            ptr += n_in
            lq = [idmap[v] for v in qry_ids[rows]]
            qT[:D, ti * 128 : ti * 128 + n_in] = qry_reps[rows].T
            qT[D:, ti * 128 : ti * 128 + n_in] = (ALPHA * _onehot6(lq)).T
            sel[np.arange(n_in), ti * 8 + qpos_q[rows]] = 1.0
            tsel = np.nonzero(np.isin(dids, ids))[0]
            for d in range(BD_PER):
                dtok = tsel[ddoc[tsel] == d]
                m = len(dtok)
                c0 = ti * ctile + d * t_slots
                dT[:D, c0 : c0 + m] = dreps[dtok].T
                dT[D:, c0 : c0 + m] = (ALPHA * _onehot6([idmap[v] for v in dids[dtok]])).T
        in_maps.append(
            {"qryT": _bf16(qT), "docT": _bf16(dT), "selT": sel}
        )
    return in_maps, nt, t_slots


def _assemble(inputs, results, nt):
    toks = np.zeros((Bq, Bd), dtype=np.float32)
    for core in range(NCORES):
        osb = np.asarray(results[core]["out"], np.float32)  # [64, 16*nt]
        part = np.zeros((Bq, BD_PER), dtype=np.float32)
        for g, grp in enumerate(_groups(nt)):
            for tl, qt in enumerate(grp):
                part += osb[8 * tl : 8 * tl + 8, qt * 16 : (qt + 1) * 16]
        toks[:, core * BD_PER : (core + 1) * BD_PER] = part
    cls = np.asarray(inputs["qry_cls"], np.float32) @ np.asarray(
        inputs["doc_cls"], np.float32
    ).T
    scores = toks + cls
    return scores.max(axis=0).reshape(-1).astype(np.float32)


def _ensure_ntff_hook():
    """This container's antenv lacks axon_hooks; synthesize the module and
    register the ctypes-based NTFF profile hook so trace=True works."""
    import sys
    import types

    if "antenv.axon_hooks" in sys.modules:
        return
    mod = types.ModuleType("antenv.axon_hooks")
    state = {"hook": None}
    mod.set_axon_ntff_profile_hook = lambda h: state.__setitem__("hook", h)
    mod.get_axon_ntff_profile_hook = lambda: state["hook"]
    sys.modules["antenv.axon_hooks"] = mod
    try:
        import antenv

        antenv.axon_hooks = mod
    except ImportError:
        pass
    try:
        from trn_agent_boot.trn_boot import _ntff_profile_via_ctypes

        mod.set_axon_ntff_profile_hook(
            _ntff_profile_via_ctypes("/opt/axon/libaxon_pjrt.so")
        )
    except Exception:
        pass


def run(inputs, trace=False, **kwargs):
    """Run on the 8 NeuronCores; returns (output, BassKernelResults)."""
    from concourse.bass_utils import run_bass_kernel_spmd

    if trace:
        _ensure_ntff_hook()
    in_maps, nt, t_slots = _prepare_in_maps(inputs)
    nc = _get_nc(nt, t_slots)
    res = run_bass_kernel_spmd(
        nc, in_maps, core_ids=list(range(NCORES)), trace=trace, **kwargs
    )
    return _assemble(inputs, res.results, nt), res


def kernel(**inputs) -> np.ndarray:
    out, _ = run(inputs)
    return out
